# revision 34
# baseline (speedup 1.0000x reference)
"""Trainium2 Bass kernel: conv3x3(64->128) + ReLU + conv3x3(128->128) + ReLU + maxpool2x2.

Input  x: [32, 64, 112, 112] f32; weights w1 [128,64,3,3], w2 [128,128,3,3]; biases [128].
Output: [32, 128, 56, 56] f32.

Strategy: data-parallel over batch across 8 cores (4 images/core). Per image,
channels live on SBUF partitions and spatial positions on the free dim with a
zero-padded 114x114 layout. Each conv tap (ky,kx) is a matmul over channels at
a shifted spatial offset, accumulated in PSUM; the moving AP is a [4,112]
strided view so the 2 pad columns per row are never streamed. Conv1 (K=64)
packs two K=64 matmuls in the 128x128 PE array via row-group tile_position
(0,0)/(64,0): the image's top/bottom row-halves are processed concurrently
from partition halves 0:64 / 64:128. Conv2 is K=128 full-array. Matmuls run
in bf16 (PSUM accumulates f32; rel err ~0.25%). ReLU+bias fused in ScalarE
PSUM->SBUF copies; maxpool via two strided VectorE max ops.
"""
import ml_dtypes
import numpy as np

import concourse.bass as bass
import concourse.mybir as mybir
from concourse import bacc
from concourse.tile import TileContext
from concourse.bass_utils import run_bass_kernel_spmd

N_CORES = 8
B, CIN, COUT, H, W = 32, 64, 128, 112, 112
PB = B // N_CORES            # images per core
HP = H + 2                   # padded width/height (114)
G = 128                      # zero guard columns around the y1 padded buffer
RHALF = 58                   # padded rows held per half-region (incl. 1-row halo)
LHALF = RHALF * HP           # 6612
XGUARD = 16                  # read-span guard after the x buffer
LY1 = G + HP * HP + G       # conv1 output (padded) buffer length
NROW = 4                     # output rows per PSUM chunk
NCH = NROW * HP              # padded span per chunk (456)
NCW = NROW * W               # matmul free dim per chunk (448, pad cols skipped)
NR1 = (H // 2) // NROW       # conv1 chunk rounds per half (14)
NR2 = H // NROW              # conv2 chunks (28)
HO, WO = H // 2, W // 2      # pooled output dims

F32 = mybir.dt.float32
BF16 = mybir.dt.bfloat16
RELU = mybir.ActivationFunctionType.Relu

# tap offsets in padded flat coords, tap t = (ky, kx)
TAP_OFF = [(ky - 1) * HP + (kx - 1) for ky in range(3) for kx in range(3)]

_CACHE = {}

TRACE = False          # test harness may flip this for profiled runs
LAST_RESULT = None     # stashes BassKernelResults of the last run


def _build():
    nc = bacc.Bacc("TRN2", target_bir_lowering=False, debug=False,
                   num_devices=N_CORES, num_swdge_queues=4)
    # x arrives host-pre-padded in the exact SBUF layout: per image
    # [128 partitions, 58 padded rows x 114 cols] with halos/borders baked
    # in, so every load is a fully contiguous per-partition DMA.
    x = nc.dram_tensor("x", [PB, 128, LHALF], BF16, kind="ExternalInput")
    w1t = nc.dram_tensor("w1t", [128, 9 * 128], BF16, kind="ExternalInput")
    w2t = nc.dram_tensor("w2t", [128, 9 * 128], BF16, kind="ExternalInput")
    b1 = nc.dram_tensor("b1", [128, 1], F32, kind="ExternalInput")
    b2 = nc.dram_tensor("b2", [128, 1], F32, kind="ExternalInput")
    y = nc.dram_tensor("y", [PB, COUT, HO, WO], F32, kind="ExternalOutput")

    with TileContext(nc) as tc:
        with (
            tc.tile_pool(name="const", bufs=1) as cpool,
            tc.tile_pool(name="xs", bufs=1) as xpool,
            tc.tile_pool(name="y1p", bufs=1) as y1pool,
            tc.tile_pool(name="work", bufs=4) as wpool,
            tc.tile_pool(name="oimg", bufs=2) as opool,
            tc.tile_pool(name="psA", bufs=2, space="PSUM") as psApool,
            tc.tile_pool(name="psB", bufs=2, space="PSUM") as psBpool,
            tc.tile_pool(name="psC", bufs=3, space="PSUM") as psCpool,
        ):
            w1sb = cpool.tile([128, 9 * 128], BF16, tag="w1")
            w2sb = cpool.tile([128, 9 * 128], BF16, tag="w2")
            b1sb = cpool.tile([128, 1], F32, tag="b1")
            b2sb = cpool.tile([128, 1], F32, tag="b2")
            # w1 gates conv1 round 0. The scalar queue delivers its first
            # packets earliest (~8.9us; gpsimd's queue has a ~3.5us start
            # latency), so w1 leads scalar, split so taps 0-2 land first --
            # round 0 consumes taps slower than the rest streams in. w2 is
            # queued on gpsimd behind image-0's x (needed only ~25us in).
            # Biases ride sync (slow 4B/partition packets, but they land
            # before the first ACTIVATE).
            nc.scalar.dma_start(out=w1sb[:, 0:384], in_=w1t[:, 0:384])
            nc.scalar.dma_start(out=w1sb[:, 384:1152], in_=w1t[:, 384:1152])
            nc.sync.dma_start(out=b1sb[:, :], in_=b1[:, :])
            nc.sync.dma_start(out=b2sb[:, :], in_=b2[:, :])

            # persistent padded buffers; x borders are host-baked, y1
            # borders are zeroed once (interior fully overwritten per image).
            xs = [xpool.tile([128, LHALF + XGUARD], BF16,
                             tag=f"xs{i}", name=f"xs{i}") for i in range(2)]
            y1 = y1pool.tile([128, LY1], BF16, tag="y1")

            def x_load(b, xsb):
                # contiguous row-span chunks; round ri reads padded rows
                # 4ri..4ri+5. Image 0 splits into 4 chunks across the
                # gpsimd + scalar queues so round-0 rows land first; later
                # images have ~70us of slack and use 2 gpsimd chunks.
                # NOTE: the sync queue issues strided DMA descriptors ~10x
                # slower than gpsimd/scalar — keep x off it.
                if b == 0:
                    for q, (r0, r1) in zip(
                            (nc.gpsimd, nc.gpsimd, nc.scalar, nc.scalar),
                            ((0, 7), (7, 19), (19, 36), (36, 58))):
                        q.dma_start(out=xsb[:, r0 * HP:r1 * HP],
                                    in_=x[b, :, r0 * HP:r1 * HP])
                else:
                    for r0, r1 in ((0, 29), (29, 58)):
                        nc.gpsimd.dma_start(out=xsb[:, r0 * HP:r1 * HP],
                                            in_=x[b, :, r0 * HP:r1 * HP])

            warm = cpool.tile([64, 64], BF16, tag="warm")
            nc.vector.memset(warm[:, :], 0.0)
            nc.vector.memset(xs[0][:, LHALF:LHALF + XGUARD], 0.0)
            nc.vector.memset(xs[1][:, LHALF:LHALF + XGUARD], 0.0)

            x_load(0, xs[0])
            nc.gpsimd.dma_start(out=w2sb[:, :], in_=w2t[:, :])

            # PE warmup: short zero matmul ticks into a throwaway PSUM tile
            # keep the PE clock gate (HAM) ramping while the initial DMAs
            # run; 64-col ticks so real round-0 work starts promptly (within
            # one tick) once w1 + the first x chunk land.
            warm_ps = psApool.tile([128, NCW], F32, tag="psA",
                                   name="warm_ps")
            N_WARM = 45
            for k in range(N_WARM):
                nc.tensor.matmul(warm_ps[0:64, 0:64], warm[:, :], warm[:, :],
                                 start=(k == 0), stop=(k == N_WARM - 1),
                                 tile_position=(0, 0))

            y1f = y1[:, :]
            nc.vector.memset(y1f[:, G - 8:G], 0.0)
            nc.vector.memset(y1f[:, G + HP * HP:G + HP * HP + 8], 0.0)
            nc.vector.memset(y1f[:, G:G + HP], 0.0)
            nc.vector.memset(y1f[:, G + 113 * HP:G + 114 * HP], 0.0)
            y1cb = y1f[:, G + 113:G + 113 + 113 * HP].rearrange(
                "p (r c) -> p r c", c=HP)
            nc.vector.memset(y1cb[:, :, 0:2], 0.0)

            y1v = y1[:, G:G + HP * HP].rearrange("p (r c) -> p r c", c=HP)

            def mv(tile, p0, p1, base):
                # [p, 4, 112] strided moving view: 4 output rows' inputs,
                # skipping the 2 pad columns per 114-wide padded row
                return tile[p0:p1, base:base + NCH].rearrange(
                    "p (r c) -> p r c", c=HP)[:, :, 0:112]

            for b in range(PB):
                xsb = xs[b % 2]
                if b > 0:
                    x_load(b, xsb)

                # ---- conv1: two concurrent K=64 row-group matmul series ----
                for ri in range(NR1):
                    r = 1 + NROW * ri          # local output row base (both halves)
                    q = r * HP
                    psA = psApool.tile([128, NCW], F32, tag="psA")
                    psB = psBpool.tile([128, NCW], F32, tag="psB")
                    for t in range(9):
                        off = TAP_OFF[t]
                        nc.tensor.matmul(psA[:, :],
                                         w1sb[0:64, t * 128:(t + 1) * 128],
                                         mv(xsb, 0, 64, q + off + 1),
                                         start=(t == 0), stop=(t == 8),
                                         tile_position=(0, 0))
                        nc.tensor.matmul(psB[:, :],
                                         w1sb[64:128, t * 128:(t + 1) * 128],
                                         mv(xsb, 64, 128, q + off + 1),
                                         start=(t == 0), stop=(t == 8),
                                         tile_position=(64, 0))
                    pAv = psA.rearrange("p (r c) -> p r c", c=W)
                    pBv = psB.rearrange("p (r c) -> p r c", c=W)
                    # top half outputs: padded rows r..r+3; bottom: 56+r..56+r+3
                    nc.scalar.activation(y1v[:, r:r + NROW, 1:113],
                                         pAv[:, :, :], RELU,
                                         bias=b1sb[:, 0:1])
                    nc.scalar.activation(y1v[:, 56 + r:56 + r + NROW, 1:113],
                                         pBv[:, :, :], RELU,
                                         bias=b1sb[:, 0:1])

                # ---- conv2 (K=128) + fused relu + maxpool ----
                out_img = opool.tile([128, HO * WO], F32, tag="oimg")

                def conv2_chunk(r, nrow, po):
                    # conv output padded rows r..r+nrow -> pooled rows
                    # po..po+nrow/2 of out_img. Pool tiles keep the full
                    # NROW shapes (sliced) so tags/banks stay uniform.
                    q = G + r * HP
                    nw = nrow * W
                    psC = psCpool.tile([128, NCW], F32, tag="psC")
                    for t in range(9):
                        base = q + TAP_OFF[t] + 1
                        mvw = y1[:, base:base + nrow * HP].rearrange(
                            "p (r c) -> p r c", c=HP)[:, :, 0:112]
                        nc.tensor.matmul(psC[:, 0:nw],
                                         w2sb[:, t * 128:(t + 1) * 128],
                                         mvw, start=(t == 0), stop=(t == 8))
                    y2c = wpool.tile([128, NCW], F32, tag="y2c")
                    nc.scalar.activation(y2c[:, 0:nw], psC[:, 0:nw], RELU,
                                         bias=b2sb[:, 0:1])
                    # horizontal 2:1 max
                    hpt = wpool.tile([128, NROW * WO], F32, tag="hp")
                    y2p = y2c[:, 0:nw].rearrange("p (r c two) -> p r c two",
                                                 two=2, c=WO)
                    nc.vector.tensor_max(
                        hpt[:, 0:nrow * WO].rearrange("p (r c) -> p r c",
                                                      c=WO),
                        y2p[:, :, :, 0], y2p[:, :, :, 1])
                    # vertical 2:1 max -> nrow/2 pooled rows
                    hpv = hpt[:, 0:nrow * WO].rearrange(
                        "p (r two c) -> p r two c", two=2, c=WO)
                    ov = out_img[:, po * WO:(po + nrow // 2) * WO].rearrange(
                        "p (r c) -> p r c", c=WO)
                    nc.vector.tensor_max(ov, hpv[:, :, 0, :], hpv[:, :, 1, :])

                def out_slab(lo_row, hi_row):
                    lo, hi = lo_row * WO, hi_row * WO
                    nc.sync.dma_start(
                        out=y[b].rearrange("c h w -> c (h w)")[:, lo:hi],
                        in_=out_img[:, lo:hi])

                for ci in range(NR2 - 1):
                    conv2_chunk(1 + NROW * ci, NROW, ci * 2)
                    if ci in (6, 13, 20, 26):
                        starts = {6: 0, 13: 14, 20: 28, 26: 42}
                        out_slab(starts[ci], (ci + 1) * 2)
                # final chunk split in two so the tail activation+pool+DMA
                # chain after the very last matmul is half as long
                conv2_chunk(109, 2, 54)
                out_slab(54, 55)
                conv2_chunk(111, 2, 55)
                out_slab(55, 56)

    nc.compile()
    return nc


def kernel(x, w1, b1, w2, b2):
    global LAST_RESULT
    x = np.ascontiguousarray(np.asarray(x, dtype=np.float32))
    w1 = np.asarray(w1, dtype=np.float32)
    w2 = np.asarray(w2, dtype=np.float32)
    b1 = np.asarray(b1, dtype=np.float32)
    b2 = np.asarray(b2, dtype=np.float32)

    if "nc" not in _CACHE:
        _CACHE["nc"] = _build()
    nc = _CACHE["nc"]

    # weight layout: w1t[ci, t*128+co] = w1[co, ci, ky, kx]; duplicated on
    # partitions 64:128 for the upper row-group. w2t likewise (full 128 rows).
    w1r = np.transpose(w1, (1, 2, 3, 0)).reshape(CIN, 9 * 128)  # ci,(ky kx co)
    # reorder to (t*128 + co): currently (ky,kx) major over co -> already t-major
    w1full = np.concatenate([w1r, w1r], axis=0)                  # [128, 1152]
    w2r = np.transpose(w2, (1, 2, 3, 0)).reshape(COUT, 9 * 128)
    # pre-pad x into the exact SBUF layout: partitions 0:64 hold the top
    # half-image (padded rows 0..57 = x rows -1..56), partitions 64:128 the
    # bottom half (x rows 55..112); borders/halos zero, cols 1:113 = data.
    xp = np.zeros((B, 128, RHALF, HP), np.float32)
    xp[:, 0:64, 1:58, 1:113] = x[:, :, 0:57, :]
    xp[:, 64:128, 0:57, 1:113] = x[:, :, 55:112, :]
    xb = xp.reshape(B, 128, LHALF).astype(ml_dtypes.bfloat16)
    w1b = np.ascontiguousarray(w1full.astype(ml_dtypes.bfloat16))
    w2b = np.ascontiguousarray(w2r.astype(ml_dtypes.bfloat16))

    in_maps = []
    for c in range(N_CORES):
        in_maps.append({
            "x": np.ascontiguousarray(xb[c * PB:(c + 1) * PB]),
            "w1t": w1b,
            "w2t": w2b,
            "b1": b1.reshape(128, 1),
            "b2": b2.reshape(128, 1),
        })

    res = run_bass_kernel_spmd(nc, in_maps, core_ids=list(range(N_CORES)),
                               trace=TRACE)
    LAST_RESULT = res
    out = np.empty((B, COUT, HO, WO), dtype=np.float32)
    for c in range(N_CORES):
        out[c * PB:(c + 1) * PB] = res.results[c]["y"]
    return out


# revision 35
# speedup vs baseline: 1.1899x; 1.1899x over previous
"""Trainium2 Bass kernel: conv3x3(64->128) + ReLU + conv3x3(128->128) + ReLU + maxpool2x2.

Input  x: [32, 64, 112, 112] f32; weights w1 [128,64,3,3], w2 [128,128,3,3]; biases [128].
Output: [32, 128, 56, 56] f32.

Strategy: data-parallel over batch across 8 cores (4 images/core). Per image,
channels live on SBUF partitions and spatial positions on the free dim with a
zero-padded 114x114 layout. Each conv tap (ky,kx) is a matmul over channels at
a shifted spatial offset, accumulated in PSUM; the moving AP is a [4,112]
strided view so the 2 pad columns per row are never streamed. Conv1 (K=64)
packs two K=64 matmuls in the 128x128 PE array via row-group tile_position
(0,0)/(64,0): the image's top/bottom row-halves are processed concurrently
from partition halves 0:64 / 64:128. Conv2 is K=128 full-array. Matmuls run
in bf16 (PSUM accumulates f32; rel err ~0.25%). ReLU+bias fused in ScalarE
PSUM->SBUF copies; maxpool via two strided VectorE max ops.
"""
import ml_dtypes
import numpy as np

import concourse.bass as bass
import concourse.mybir as mybir
from concourse import bacc
from concourse.tile import TileContext
from concourse.bass_utils import run_bass_kernel_spmd

N_CORES = 8
B, CIN, COUT, H, W = 32, 64, 128, 112, 112
PB = B // N_CORES            # images per core
HP = H + 2                   # padded width/height (114)
G = 128                      # zero guard columns around the y1 padded buffer
RHALF = 58                   # padded rows held per half-region (incl. 1-row halo)
LHALF = RHALF * HP           # 6612
XGUARD = 16                  # read-span guard after the x buffer
LY1 = G + HP * HP + G       # conv1 output (padded) buffer length
NROW = 4                     # output rows per PSUM chunk
NCH = NROW * HP              # padded span per chunk (456)
NCW = NROW * W               # matmul free dim per chunk (448, pad cols skipped)
NR1 = (H // 2) // NROW       # conv1 chunk rounds per half (14)
NR2 = H // NROW              # conv2 chunks (28)
HO, WO = H // 2, W // 2      # pooled output dims

F32 = mybir.dt.float32
BF16 = mybir.dt.bfloat16
RELU = mybir.ActivationFunctionType.Relu

# tap offsets in padded flat coords, tap t = (ky, kx)
TAP_OFF = [(ky - 1) * HP + (kx - 1) for ky in range(3) for kx in range(3)]

_CACHE = {}

TRACE = False          # test harness may flip this for profiled runs
LAST_RESULT = None     # stashes BassKernelResults of the last run


def _build():
    nc = bacc.Bacc("TRN2", target_bir_lowering=False, debug=False,
                   num_devices=N_CORES, num_swdge_queues=4)
    # x arrives host-pre-padded in the exact SBUF layout: per image
    # [128 partitions, 58 padded rows x 114 cols] with halos/borders baked
    # in, so every load is a fully contiguous per-partition DMA.
    x = nc.dram_tensor("x", [PB, 128, LHALF], BF16, kind="ExternalInput")
    w1t = nc.dram_tensor("w1t", [128, 9 * 128], BF16, kind="ExternalInput")
    w2t = nc.dram_tensor("w2t", [128, 9 * 128], BF16, kind="ExternalInput")
    b1 = nc.dram_tensor("b1", [128, 1], F32, kind="ExternalInput")
    b2 = nc.dram_tensor("b2", [128, 1], F32, kind="ExternalInput")
    y = nc.dram_tensor("y", [PB, COUT, HO, WO], F32, kind="ExternalOutput")

    with TileContext(nc) as tc:
        with (
            tc.tile_pool(name="const", bufs=1) as cpool,
            tc.tile_pool(name="xs", bufs=1) as xpool,
            tc.tile_pool(name="y1p", bufs=1) as y1pool,
            tc.tile_pool(name="work", bufs=4) as wpool,
            tc.tile_pool(name="oimg", bufs=2) as opool,
            tc.tile_pool(name="psA", bufs=2, space="PSUM") as psApool,
            tc.tile_pool(name="psB", bufs=2, space="PSUM") as psBpool,
            tc.tile_pool(name="psC", bufs=3, space="PSUM") as psCpool,
        ):
            w1sb = cpool.tile([128, 9 * 128], BF16, tag="w1")
            w2sb = cpool.tile([128, 9 * 128], BF16, tag="w2")
            b1sb = cpool.tile([128, 1], F32, tag="b1")
            b2sb = cpool.tile([128, 1], F32, tag="b2")
            # w1 gates conv1 round 0. The scalar queue delivers its first
            # packets earliest (~8.9us; gpsimd's queue has a ~3.5us start
            # latency), so w1 leads scalar as ONE dma: splitting it leaves
            # two writers on w1sb and every conv1 LDWEIGHTS then carries two
            # semaphore waits forever (+35ns per tap-pair, measured). w2 is
            # queued on gpsimd behind image-0's x (needed only ~25us in).
            # Biases ride sync (slow 4B/partition packets, but they land
            # before the first ACTIVATE).
            nc.scalar.dma_start(out=w1sb[:, :], in_=w1t[:, :])
            nc.sync.dma_start(out=b1sb[:, :], in_=b1[:, :])
            nc.sync.dma_start(out=b2sb[:, :], in_=b2[:, :])

            # persistent padded buffers; x borders are host-baked, y1
            # borders are zeroed once (interior fully overwritten per image).
            xs = [xpool.tile([128, LHALF + XGUARD], BF16,
                             tag=f"xs{i}", name=f"xs{i}") for i in range(2)]
            y1 = y1pool.tile([128, LY1], BF16, tag="y1")

            def x_load(b, xsb):
                # contiguous row-span chunks; round ri reads padded rows
                # 4ri..4ri+5. Image 0 splits into 4 chunks across the
                # gpsimd + scalar queues so round-0 rows land first; later
                # images have ~70us of slack and use 2 gpsimd chunks.
                # NOTE: the sync queue issues strided DMA descriptors ~10x
                # slower than gpsimd/scalar — keep x off it.
                if b == 0:
                    for q, (r0, r1) in zip(
                            (nc.gpsimd, nc.gpsimd, nc.scalar, nc.scalar),
                            ((0, 7), (7, 19), (19, 36), (36, 58))):
                        q.dma_start(out=xsb[:, r0 * HP:r1 * HP],
                                    in_=x[b, :, r0 * HP:r1 * HP])
                else:
                    for r0, r1 in ((0, 29), (29, 58)):
                        nc.gpsimd.dma_start(out=xsb[:, r0 * HP:r1 * HP],
                                            in_=x[b, :, r0 * HP:r1 * HP])

            warm = cpool.tile([64, 64], BF16, tag="warm")
            nc.vector.memset(warm[:, :], 0.0)
            nc.vector.memset(xs[0][:, LHALF:LHALF + XGUARD], 0.0)
            nc.vector.memset(xs[1][:, LHALF:LHALF + XGUARD], 0.0)

            x_load(0, xs[0])
            nc.gpsimd.dma_start(out=w2sb[:, :], in_=w2t[:, :])

            # PE warmup: short zero matmul ticks into a throwaway PSUM tile
            # keep the PE clock gate (HAM) ramping while the initial DMAs
            # run; 64-col ticks so real round-0 work starts promptly (within
            # one tick) once w1 + the first x chunk land.
            warm_ps = psApool.tile([128, NCW], F32, tag="psA",
                                   name="warm_ps")
            N_WARM = 45
            for k in range(N_WARM):
                nc.tensor.matmul(warm_ps[0:64, 0:64], warm[:, :], warm[:, :],
                                 start=(k == 0), stop=(k == N_WARM - 1),
                                 tile_position=(0, 0))

            y1f = y1[:, :]
            nc.vector.memset(y1f[:, G - 8:G], 0.0)
            nc.vector.memset(y1f[:, G + HP * HP:G + HP * HP + 8], 0.0)
            nc.vector.memset(y1f[:, G:G + HP], 0.0)
            nc.vector.memset(y1f[:, G + 113 * HP:G + 114 * HP], 0.0)
            y1cb = y1f[:, G + 113:G + 113 + 113 * HP].rearrange(
                "p (r c) -> p r c", c=HP)
            nc.vector.memset(y1cb[:, :, 0:2], 0.0)

            y1v = y1[:, G:G + HP * HP].rearrange("p (r c) -> p r c", c=HP)

            def mv(tile, p0, p1, base):
                # [p, 4, 112] strided moving view: 4 output rows' inputs,
                # skipping the 2 pad columns per 114-wide padded row
                return tile[p0:p1, base:base + NCH].rearrange(
                    "p (r c) -> p r c", c=HP)[:, :, 0:112]

            for b in range(PB):
                xsb = xs[b % 2]
                if b > 0:
                    x_load(b, xsb)

                # ---- conv1: two concurrent K=64 row-group matmul series ----
                for ri in range(NR1):
                    r = 1 + NROW * ri          # local output row base (both halves)
                    q = r * HP
                    psA = psApool.tile([128, NCW], F32, tag="psA")
                    psB = psBpool.tile([128, NCW], F32, tag="psB")
                    for t in range(9):
                        off = TAP_OFF[t]
                        nc.tensor.matmul(psA[:, :],
                                         w1sb[0:64, t * 128:(t + 1) * 128],
                                         mv(xsb, 0, 64, q + off + 1),
                                         start=(t == 0), stop=(t == 8),
                                         tile_position=(0, 0))
                        nc.tensor.matmul(psB[:, :],
                                         w1sb[64:128, t * 128:(t + 1) * 128],
                                         mv(xsb, 64, 128, q + off + 1),
                                         start=(t == 0), stop=(t == 8),
                                         tile_position=(64, 0))
                    pAv = psA.rearrange("p (r c) -> p r c", c=W)
                    pBv = psB.rearrange("p (r c) -> p r c", c=W)
                    # top half outputs: padded rows r..r+3; bottom: 56+r..56+r+3
                    nc.scalar.activation(y1v[:, r:r + NROW, 1:113],
                                         pAv[:, :, :], RELU,
                                         bias=b1sb[:, 0:1])
                    nc.scalar.activation(y1v[:, 56 + r:56 + r + NROW, 1:113],
                                         pBv[:, :, :], RELU,
                                         bias=b1sb[:, 0:1])

                # ---- conv2 (K=128) + fused relu + maxpool ----
                out_img = opool.tile([128, HO * WO], F32, tag="oimg")

                def conv2_chunk(r, nrow, po):
                    # conv output padded rows r..r+nrow -> pooled rows
                    # po..po+nrow/2 of out_img. Pool tiles keep the full
                    # NROW shapes (sliced) so tags/banks stay uniform.
                    q = G + r * HP
                    nw = nrow * W
                    psC = psCpool.tile([128, NCW], F32, tag="psC")
                    for t in range(9):
                        base = q + TAP_OFF[t] + 1
                        mvw = y1[:, base:base + nrow * HP].rearrange(
                            "p (r c) -> p r c", c=HP)[:, :, 0:112]
                        nc.tensor.matmul(psC[:, 0:nw],
                                         w2sb[:, t * 128:(t + 1) * 128],
                                         mvw, start=(t == 0), stop=(t == 8))
                    y2c = wpool.tile([128, NCW], F32, tag="y2c")
                    nc.scalar.activation(y2c[:, 0:nw], psC[:, 0:nw], RELU,
                                         bias=b2sb[:, 0:1])
                    # horizontal 2:1 max
                    hpt = wpool.tile([128, NROW * WO], F32, tag="hp")
                    y2p = y2c[:, 0:nw].rearrange("p (r c two) -> p r c two",
                                                 two=2, c=WO)
                    nc.vector.tensor_max(
                        hpt[:, 0:nrow * WO].rearrange("p (r c) -> p r c",
                                                      c=WO),
                        y2p[:, :, :, 0], y2p[:, :, :, 1])
                    # vertical 2:1 max -> nrow/2 pooled rows
                    hpv = hpt[:, 0:nrow * WO].rearrange(
                        "p (r two c) -> p r two c", two=2, c=WO)
                    ov = out_img[:, po * WO:(po + nrow // 2) * WO].rearrange(
                        "p (r c) -> p r c", c=WO)
                    nc.vector.tensor_max(ov, hpv[:, :, 0, :], hpv[:, :, 1, :])

                def out_slab(lo_row, hi_row):
                    lo, hi = lo_row * WO, hi_row * WO
                    nc.sync.dma_start(
                        out=y[b].rearrange("c h w -> c (h w)")[:, lo:hi],
                        in_=out_img[:, lo:hi])

                for ci in range(NR2 - 1):
                    conv2_chunk(1 + NROW * ci, NROW, ci * 2)
                    if ci in (6, 13, 20, 26):
                        starts = {6: 0, 13: 14, 20: 28, 26: 42}
                        out_slab(starts[ci], (ci + 1) * 2)
                # final chunk split in two so the tail activation+pool+DMA
                # chain after the very last matmul is half as long
                conv2_chunk(109, 2, 54)
                out_slab(54, 55)
                conv2_chunk(111, 2, 55)
                out_slab(55, 56)

    nc.compile()
    return nc


def kernel(x, w1, b1, w2, b2):
    global LAST_RESULT
    x = np.ascontiguousarray(np.asarray(x, dtype=np.float32))
    w1 = np.asarray(w1, dtype=np.float32)
    w2 = np.asarray(w2, dtype=np.float32)
    b1 = np.asarray(b1, dtype=np.float32)
    b2 = np.asarray(b2, dtype=np.float32)

    if "nc" not in _CACHE:
        _CACHE["nc"] = _build()
    nc = _CACHE["nc"]

    # weight layout: w1t[ci, t*128+co] = w1[co, ci, ky, kx]; duplicated on
    # partitions 64:128 for the upper row-group. w2t likewise (full 128 rows).
    w1r = np.transpose(w1, (1, 2, 3, 0)).reshape(CIN, 9 * 128)  # ci,(ky kx co)
    # reorder to (t*128 + co): currently (ky,kx) major over co -> already t-major
    w1full = np.concatenate([w1r, w1r], axis=0)                  # [128, 1152]
    w2r = np.transpose(w2, (1, 2, 3, 0)).reshape(COUT, 9 * 128)
    # pre-pad x into the exact SBUF layout: partitions 0:64 hold the top
    # half-image (padded rows 0..57 = x rows -1..56), partitions 64:128 the
    # bottom half (x rows 55..112); borders/halos zero, cols 1:113 = data.
    xp = np.zeros((B, 128, RHALF, HP), np.float32)
    xp[:, 0:64, 1:58, 1:113] = x[:, :, 0:57, :]
    xp[:, 64:128, 0:57, 1:113] = x[:, :, 55:112, :]
    xb = xp.reshape(B, 128, LHALF).astype(ml_dtypes.bfloat16)
    w1b = np.ascontiguousarray(w1full.astype(ml_dtypes.bfloat16))
    w2b = np.ascontiguousarray(w2r.astype(ml_dtypes.bfloat16))

    in_maps = []
    for c in range(N_CORES):
        in_maps.append({
            "x": np.ascontiguousarray(xb[c * PB:(c + 1) * PB]),
            "w1t": w1b,
            "w2t": w2b,
            "b1": b1.reshape(128, 1),
            "b2": b2.reshape(128, 1),
        })

    res = run_bass_kernel_spmd(nc, in_maps, core_ids=list(range(N_CORES)),
                               trace=TRACE)
    LAST_RESULT = res
    out = np.empty((B, COUT, HO, WO), dtype=np.float32)
    for c in range(N_CORES):
        out[c * PB:(c + 1) * PB] = res.results[c]["y"]
    return out
